# revision 12
# baseline (speedup 1.0000x reference)
import sys

for p in ("/opt/trn_rl_repo", "/opt/pypackages"):
    if p not in sys.path:
        sys.path.insert(0, p)

import numpy as np

import concourse.bass as bass
import concourse.tile as tile
from concourse import mybir
from concourse.bass_utils import run_bass_kernel_spmd

# Problem constants (hardcoded per spec: x is (128, 14, 14, 768), 8 heads, r=4)
B, H, W, C = 128, 14, 14, 768
N = H * W            # 196 tokens
NH = 8               # heads (== k)
HD = C // NH         # 96 head dim
CR = C // 4          # 192 adapter hidden
NCORES = 8
BL = B // NCORES     # 16 batch items per core
DT = mybir.dt.float32

CHUNKS = [(0, 128), (128, 68)]   # n=196 on partitions
CKC = 6                          # input-side C chunks of 128
AF = mybir.ActivationFunctionType


def build_nc():
    nc = bass.Bass()
    x_p = nc.declare_dram_parameter("x", [BL, N, C], DT, isOutput=False)
    bank_p = nc.declare_dram_parameter("bank", [NH, N, N], DT, isOutput=False)
    a1_p = nc.declare_dram_parameter("a1t", [C + 1, CR], DT, isOutput=False)
    a2_p = nc.declare_dram_parameter("a2t", [CR + 1, NH * NH], DT, isOutput=False)
    pre_p = nc.declare_dram_parameter("pret", [C + 1, C], DT, isOutput=False)
    post_p = nc.declare_dram_parameter("postt", [C + 1, C], DT, isOutput=False)
    id_p = nc.declare_dram_parameter("ident", [128, 128], DT, isOutput=False)
    out_p = nc.declare_dram_parameter("out", [BL, N, C], DT, isOutput=True)

    with tile.TileContext(nc) as tc:
        with (
            tc.tile_pool(name="wpool", bufs=1) as wp,
            tc.tile_pool(name="xpool", bufs=2) as xpool,
            tc.tile_pool(name="spool", bufs=2) as sp,
            tc.tile_pool(name="psum", bufs=1, space="PSUM") as pp,
        ):
            # ---- stage replicated weights once (direct DMA) ----
            def stage(dst_ap, src_ap, rows, width):
                nc.sync.dma_start(dst_ap, src_ap)

            pre_t, a1_t = [], []
            for kc in range(CKC + 1):
                rows = 128 if kc < CKC else 1
                t = wp.tile([128, C], DT, tag=f"pre{kc}", name=f"pre{kc}")
                stage(t[:rows, :], pre_p[kc * 128 : kc * 128 + rows, :], rows, C)
                pre_t.append(t)
                t = wp.tile([128, CR], DT, tag=f"a1{kc}", name=f"a1{kc}")
                stage(t[:rows, :], a1_p[kc * 128 : kc * 128 + rows, :], rows, CR)
                a1_t.append(t)
            # post-side K chunks of 96 (aligned with heads)
            post_t = []
            for kc in range(NH + 1):
                rows = HD if kc < NH else 1
                t = wp.tile([128, C], DT, tag=f"post{kc}", name=f"post{kc}")
                stage(t[:rows, :], post_p[kc * HD : kc * HD + rows, :], rows, C)
                post_t.append(t)
            a2_t = []
            for kc, rows in ((0, 128), (1, 65)):
                t = wp.tile([128, NH * NH], DT, tag=f"a2{kc}", name=f"a2{kc}")
                stage(t[:rows, :], a2_p[kc * 128 : kc * 128 + rows, :], rows, NH * NH)
                a2_t.append(t)
            bank_t = {}
            for k in range(NH):
                for ci, (cs, cn) in enumerate(CHUNKS):
                    t = wp.tile([128, N], DT, tag=f"bank{k}_{ci}", name=f"bank{k}_{ci}")
                    stage(t[:cn, :], bank_p[k, cs : cs + cn, :], cn, N)
                    bank_t[(k, ci)] = t
            identr = wp.tile([128, 128], DT, tag="identr", name="identr")
            nc.sync.dma_start(identr[:, :], id_p[:, :])
            ident_a = wp.tile([128, 128], DT, tag="ident_a", name="ident_a")
            nc.scalar.copy(ident_a[:, :], identr[:, :])
            ones_row = wp.tile([1, N], DT, tag="ones_row", name="ones_row")
            nc.vector.memset(ones_row[:, :], 1.0)
            ones_col = wp.tile([128, 1], DT, tag="ones_col", name="ones_col")
            nc.vector.memset(ones_col[:, :], 1.0)

            # ---- per batch item ----
            for b in range(BL):
                xin = [
                    xpool.tile([128, C], DT, tag=f"xin{ci}", name=f"xin{ci}")
                    for ci in range(2)
                ]
                xinc = [
                    xpool.tile([128, C], DT, tag=f"xinc{ci}", name=f"xinc{ci}")
                    for ci in range(2)
                ]
                for ci, (cs, cn) in enumerate(CHUNKS):
                    nc.gpsimd.dma_start(xin[ci][:cn, :], x_p[b, cs : cs + cn, :])
                    nc.scalar.copy(xinc[ci][:cn, :], xin[ci][:cn, :])

                # transpose x -> xfT[kc]: [128, 196] for kc in 6
                xfT = []
                for kc in range(CKC):
                    t = xpool.tile([128, N], DT, tag=f"xfT{kc}", name=f"xfT{kc}")
                    for ci, (cs, cn) in enumerate(CHUNKS):
                        ps = pp.tile([128, 128], DT, tag="ps", name="ps", bufs=2)
                        nc.tensor.transpose(
                            ps[:, :cn],
                            xinc[ci][:cn, kc * 128 : (kc + 1) * 128],
                            ident_a[:cn, :cn],
                        )
                        nc.scalar.activation(t[:, cs : cs + cn], ps[:, :cn], AF.Copy)
                    xfT.append(t)

                # adapter1 -> gelu (transposed): hg [192(+ones), 196]
                hg = [
                    xpool.tile([128, N], DT, tag="hg0", name="hg0"),
                    xpool.tile([65, N], DT, tag="hg1", name="hg1"),
                ]
                for mi, (ms, mn) in enumerate(((0, 128), (128, 64))):
                    hp = pp.tile([128, N], DT, tag="hp", name="hp", bufs=1)
                    for kc in range(CKC + 1):
                        rows = 128 if kc < CKC else 1
                        rhs = xfT[kc][:, :] if kc < CKC else ones_row[:1, :]
                        nc.tensor.matmul(
                            hp[:mn, :],
                            a1_t[kc][:rows, ms : ms + mn],
                            rhs,
                            start=(kc == 0),
                            stop=(kc == CKC),
                        )
                    nc.scalar.activation(hg[mi][:mn, :], hp[:mn, :], AF.Gelu)
                nc.scalar.copy(hg[1][64:65, :], ones_row[:1, :])

                # adapter2: mixT [64, 196] then transpose to mix [n, 64]
                mp = pp.tile([128, N], DT, tag="hp", name="mp", bufs=1)
                nc.tensor.matmul(mp[:64, :], a2_t[0][:, :], hg[0][:, :], start=True, stop=False)
                nc.tensor.matmul(mp[:64, :], a2_t[1][:65, :], hg[1][:65, :], start=False, stop=True)
                mixT = xpool.tile([64, N], DT, tag="mixT", name="mixT")
                nc.scalar.activation(mixT[:, :], mp[:64, :], AF.Copy)
                mix = []
                for ci, (cs, cn) in enumerate(CHUNKS):
                    tp = pp.tile([128, 128], DT, tag="ps", name="tp", bufs=2)
                    nc.tensor.transpose(
                        tp[:cn, :64], mixT[:, cs : cs + cn], ident_a[:64, :64]
                    )
                    mt = xpool.tile([128, NH * NH], DT, tag=f"mix{ci}", name=f"mix{ci}")
                    nc.scalar.activation(mt[:cn, :], tp[:cn, :64], AF.Copy)
                    mix.append(mt)

                # pre-projection xpv[ci]: [cn, 768]
                xpv = [
                    xpool.tile([128, C], DT, tag="xp0", name="xp0"),
                    xpool.tile([68, C], DT, tag="xp1", name="xp1"),
                ]
                for ci, (cs, cn) in enumerate(CHUNKS):
                    for h2 in range(2):
                        acc = pp.tile([128, 384], DT, tag="acc", name="acc", bufs=2)
                        for kc in range(CKC + 1):
                            lhsT = (
                                xfT[kc][:, cs : cs + cn]
                                if kc < CKC
                                else ones_row[:1, cs : cs + cn]
                            )
                            rows = 128 if kc < CKC else 1
                            nc.tensor.matmul(
                                acc[:cn, :],
                                lhsT,
                                pre_t[kc][:rows, h2 * 384 : (h2 + 1) * 384],
                                start=(kc == 0),
                                stop=(kc == CKC),
                            )
                        nc.scalar.activation(
                            xpv[ci][:cn, h2 * 384 : (h2 + 1) * 384], acc[:cn, :], AF.Copy
                        )

                # scores + exp: E[(ci,h)]: [cn, 196]
                E = {}
                for ci, (cs, cn) in enumerate(CHUNKS):
                    for h in range(NH):
                        st = sp.tile([128, N], DT, tag="st", name="st", bufs=2)
                        tt = sp.tile([128, N], DT, tag="tt", name="tt", bufs=2)
                        for k in range(NH):
                            col = mix[ci][:cn, k * NH + h : k * NH + h + 1]
                            dst = st if k == 0 else tt
                            nc.scalar.activation(
                                dst[:cn, :], bank_t[(k, ci)][:cn, :], AF.Copy, scale=col
                            )
                            if k > 0:
                                nc.vector.tensor_add(st[:cn, :], st[:cn, :], tt[:cn, :])
                        e = sp.tile(
                            [128, N], DT, tag=f"E{ci}_{h}", name=f"E{ci}_{h}", bufs=2
                        )
                        nc.scalar.activation(e[:cn, :], st[:cn, :], AF.Exp)
                        E[(ci, h)] = e

                # attention + normalize: outm[(mi,h)]: [mn, 96]
                outm = {}
                for h in range(NH):
                    for mi, (ms, mn) in enumerate(CHUNKS):
                        up = pp.tile([128, HD], DT, tag="up", name="up", bufs=2)
                        zp = pp.tile([128, 1], DT, tag="zp", name="zp", bufs=1)
                        for ci, (cs, cn) in enumerate(CHUNKS):
                            lhsT = E[(ci, h)][:cn, ms : ms + mn]
                            nc.tensor.matmul(
                                up[:mn, :],
                                lhsT,
                                xpv[ci][:cn, h * HD : (h + 1) * HD],
                                start=(ci == 0),
                                stop=(ci == 1),
                            )
                            nc.tensor.matmul(
                                zp[:mn, :],
                                lhsT,
                                ones_col[:cn, :],
                                start=(ci == 0),
                                stop=(ci == 1),
                            )
                        rz = sp.tile([128, 1], DT, tag="rz", name="rz", bufs=2)
                        nc.vector.reciprocal(rz[:mn, :], zp[:mn, :])
                        om = xpool.tile(
                            [128, HD], DT, tag=f"om{mi}_{h}", name=f"om{mi}_{h}"
                        )
                        nc.scalar.activation(
                            om[:mn, :], up[:mn, :], AF.Copy, scale=rz[:mn, :]
                        )
                        outm[(mi, h)] = om

                # transpose outm -> outT[kc]: [96, 196] per head-chunk kc
                outT = []
                for kc in range(NH):
                    t = xpool.tile([HD, N], DT, tag=f"outT{kc}", name=f"outT{kc}")
                    for mi, (ms, mn) in enumerate(CHUNKS):
                        ps2 = pp.tile([128, 128], DT, tag="ps", name="ps2", bufs=2)
                        nc.tensor.transpose(
                            ps2[:HD, :mn], outm[(mi, kc)][:mn, :], ident_a[:mn, :mn]
                        )
                        nc.scalar.activation(t[:, ms : ms + mn], ps2[:HD, :mn], AF.Copy)
                    outT.append(t)

                # post-projection -> ofin[ci] -> DRAM
                ofin = [
                    xpool.tile([128, C], DT, tag="of0", name="of0"),
                    xpool.tile([68, C], DT, tag="of1", name="of1"),
                ]
                for ci, (cs, cn) in enumerate(CHUNKS):
                    for h2 in range(2):
                        acc2 = pp.tile([128, 384], DT, tag="acc", name="acc2", bufs=2)
                        for kc in range(NH + 1):
                            lhsT = (
                                outT[kc][:, cs : cs + cn]
                                if kc < NH
                                else ones_row[:1, cs : cs + cn]
                            )
                            rows = HD if kc < NH else 1
                            nc.tensor.matmul(
                                acc2[:cn, :],
                                lhsT,
                                post_t[kc][:rows, h2 * 384 : (h2 + 1) * 384],
                                start=(kc == 0),
                                stop=(kc == NH),
                            )
                        nc.scalar.activation(
                            ofin[ci][:cn, h2 * 384 : (h2 + 1) * 384], acc2[:cn, :], AF.Copy
                        )
                for ci, (cs, cn) in enumerate(CHUNKS):
                    nc.gpsimd.dma_start(out_p[b, cs : cs + cn, :], ofin[ci][:cn, :])
    _strip_redundant_dma_waits(nc)
    return nc


def _strip_redundant_dma_waits(nc):
    # This walrus build allows one sync-wait slot per instruction (two for
    # non-transpose Matmult: LDW+MM). Hoist excess waits into standalone
    # EventSemaphore instructions on the same engine, placed just before.
    import bass_rust

    f = nc.m.functions[0]
    cnt = 0
    for bb in f.blocks:
        il = bb.instructions
        out = []
        changed = False
        for ins in il:
            si = ins.sync_info
            if si is None:
                out.append(ins)
                continue
            waits = list(si.on_wait)
            limit = 1
            if len(waits) > limit:
                for w in waits[:-limit]:
                    cnt += 1
                    out.append(
                        mybir.InstEventSemaphore(
                            name=f"hoistw{cnt}",
                            engine=ins.engine,
                            debug=ins.debug,
                            sync_info=bass_rust.SyncInfo(on_wait=[w], on_update=[]),
                        )
                    )
                si.on_wait = waits[-limit:]
                changed = True
            out.append(ins)
        if changed:
            il[:] = out


_NC = None


def kernel(**inputs):
    global _NC
    x = np.ascontiguousarray(inputs["x"], dtype=np.float32).reshape(B, N, C)
    wb = np.asarray(inputs["weight_bank"], dtype=np.float32)
    rel = np.asarray(inputs["rel_idx"]).reshape(-1)
    bank = np.ascontiguousarray(wb[:, rel].reshape(NH, N, N))
    a1t = np.ascontiguousarray(
        np.vstack([np.asarray(inputs["a1_w"], np.float32).T,
                   np.asarray(inputs["a1_b"], np.float32)[None, :]])
    )
    a2t = np.ascontiguousarray(
        np.vstack([np.asarray(inputs["a2_w"], np.float32).T,
                   np.asarray(inputs["a2_b"], np.float32)[None, :]])
    )
    pret = np.ascontiguousarray(
        np.vstack([np.asarray(inputs["pre_w"], np.float32).T,
                   np.asarray(inputs["pre_b"], np.float32)[None, :]])
    )
    postt = np.ascontiguousarray(
        np.vstack([np.asarray(inputs["post_w"], np.float32).T,
                   np.asarray(inputs["post_b"], np.float32)[None, :]])
    )
    ident = np.eye(128, dtype=np.float32)

    if _NC is None:
        _NC = build_nc()

    in_maps = []
    for i in range(NCORES):
        in_maps.append(
            {
                "x": np.ascontiguousarray(x[i * BL : (i + 1) * BL]),
                "bank": bank,
                "a1t": a1t,
                "a2t": a2t,
                "pret": pret,
                "postt": postt,
                "ident": ident,
            }
        )
    res = run_bass_kernel_spmd(_NC, in_maps, list(range(NCORES)))
    out = np.concatenate([res.results[i]["out"] for i in range(NCORES)], axis=0)
    return out.reshape(B, H, W, C).astype(np.float32)
